# revision 31
# baseline (speedup 1.0000x reference)
"""BiGRU (S=512, B=64, I=256, H=512, L=2) Trainium2 Bass kernel.

Strategy: 4-way batch split x 2-way direction split across 8 NeuronCores.
Cores 0-3 run the forward GRU chain (layers 0 and 1) for batch quarters
0-3; cores 4-7 run the backward chain (fed time-reversed input, so the
device program is identical on every core).  Per layer each core does:

  P-phase: gxT = Wih @ xT + bias  (big efficient matmul, bf16, weights
           stationary, all timesteps as the moving operand), written to
           DRAM in a scan-friendly blocked layout.
  S-phase: 512-step sequential GRU scan.  Per step the 48 Whh weight
           tiles stream through the PE; the moving operand is the PAIR
           (m, p1) with h = m + p1 (linearity of the recurrent matmul),
           so the final h combine is off the critical path (done lazily
           on GpSimd one step later).  Gate math chain:
             z  = sigmoid(ghz)         omz = sigmoid(-ghz)
             p1 = z * h_prev           r   = sigmoid(ghr)
             t  = tanh(r*ghn + gxn)  (split in two fold-halves)
             m  = omz * t
           The n-gate PSUM is split in two half tiles so the tanh chain
           starts as soon as the first half's matmuls land.

Between layers the forward/backward partners exchange their hidden-state
sequences with pairwise AllGathers split into 4 time-chunks, each fired
as soon as its chunk of y0ex is written during the scan (overlapping the
collective with the scan).  Final un-transpose / un-reverse of the
output happens on the host.
"""

import os
import sys
import numpy as np

for _p in ("/opt/trn_rl_repo", "/root/.axon_site/_ro/trn_rl_repo"):
    if os.path.isdir(_p) and _p not in sys.path:
        sys.path.insert(0, _p)

import ml_dtypes
from contextlib import ExitStack

import concourse.bass as bass
import concourse.tile as tile
from concourse import bacc, mybir
from concourse.bass import ts
from concourse.bass_utils import run_bass_kernel_spmd

BF16 = mybir.dt.bfloat16
FP8 = mybir.dt.float8e4
F32 = mybir.dt.float32
AF = mybir.ActivationFunctionType
ALU = mybir.AluOpType

# Whh is stored in fp8-e4m3 scaled so max|W| -> 240; the descale folds into
# the activation `scale` operand (gates) / one fused scalar_tensor_tensor (n).
WSCALE = float(240.0 * np.sqrt(512.0))
SINV = float(1.0 / WSCALE)

S, B, I, H, L = 512, 64, 256, 512, 2
G = 3 * H            # 1536 gate rows (r, z, n)
NCORE = 8
BQ = B // 4          # 16 batch per core
SB = S * BQ          # 8192 moving columns
F = H // 128         # 4 h-fold chunks
M12 = G // 128       # 12 gate chunks
KI0 = I // 128       # 2 contraction chunks, layer-0 input proj
KI1 = 2 * H // 128   # 8 contraction chunks, layer-1 input proj
NCOL = 512           # P-phase moving chunk width
TBLK = 16            # gx prefetch / y writeback block (steps)
NB = S // TBLK       # 32 blocks
NCH = 8              # AllGather chunks
NBC = NB // NCH      # blocks per chunk
SBC = SB // NCH      # columns per chunk
UB = NCOL // BQ      # steps covered by one P-phase column chunk (32)


class PStream:
    """Input-projection tile stream: gx = W @ xT + bias, bf16, written to
    gx_dram in blocked layout [128, NB, M12, TBLK*BQ].  Tiles can be emitted
    in bulk (emit_chunks/finish) or one at a time (emit_tile) so they
    interleave into the scan's idle PE slots."""

    def __init__(self, ctx, tc, nc, wT_dram, gbias_dram, gx_dram, ki, rhs_fn,
                 tag, psum_bufs=4):
        self.nc = nc
        self.ki = ki
        self.rhs_fn = rhs_fn
        self.wpool = ctx.enter_context(tc.tile_pool(name=f"w_{tag}", bufs=1))
        self.bpool = ctx.enter_context(tc.tile_pool(name=f"b_{tag}", bufs=1))
        self.psum = ctx.enter_context(
            tc.tile_pool(name=f"ps_{tag}", bufs=psum_bufs, space="PSUM"))
        self.stg = ctx.enter_context(tc.tile_pool(name=f"st_{tag}", bufs=4))

        self.wsb = self.wpool.tile([128, ki, G], BF16)
        nc.sync.dma_start(self.wsb[:],
                          wT_dram.ap().rearrange("(k p) g -> p k g", p=128))
        self.gb = self.bpool.tile([128, M12], F32)
        nc.sync.dma_start(self.gb[:], gbias_dram.ap())

        self.gx_r = gx_dram.ap().rearrange("p (blk m c) -> p blk m c",
                                           m=M12, c=TBLK * BQ)
        self.nub = NCOL // (TBLK * BQ)  # u-blocks per column chunk (2)
        self.tiles = [(c, m) for c in range(SB // NCOL) for m in range(M12)]
        self.pos = 0
        self.rhs_tiles = None

    def emit_tile(self):
        if self.pos >= len(self.tiles):
            return False
        nc_ = self.nc
        c, m = self.tiles[self.pos]
        self.pos += 1
        if m == 0:
            self.rhs_tiles = self.rhs_fn(c)  # list of ki [128, NCOL] bf16 APs
        ps = self.psum.tile([128, NCOL], F32)
        for k in range(self.ki):
            nc_.tensor.matmul(
                ps[:],
                lhsT=self.wsb[:, k, ts(m, 128)],
                rhs=self.rhs_tiles[k],
                start=(k == 0),
                stop=(k == self.ki - 1),
            )
        # r,z chunks (m<8) are pre-scaled by WSCALE so the S-phase can
        # descale the whole PSUM (Whh fp8 part + injected gx) at once.
        # gbias for m<8 comes pre-scaled from the host.
        out = self.stg.tile([128, NCOL], BF16)
        sc = WSCALE if m < 2 * F else 1.0
        if m % 2 == 0:
            nc_.scalar.activation(out[:], ps[:], AF.Identity,
                                  bias=self.gb[:, m: m + 1], scale=sc)
        else:
            nc_.vector.tensor_scalar(out[:], ps[:], sc, self.gb[:, m: m + 1],
                                     ALU.mult, ALU.add)
        nc_.sync.dma_start(
            self.gx_r[:, ts(c, self.nub), m, :],
            out[:].rearrange("p (i c) -> p i c", c=TBLK * BQ),
        )
        return True

    def emit_chunks(self, n):
        for _ in range(n * M12):
            self.emit_tile()

    def finish(self):
        while self.emit_tile():
            pass


def _s_phase(ctx, tc, nc, whhT_dram, nbias_dram, gx_dram, layer, y0own,
             y1T_dram, ident_dram, y0ex_chunks, ag_fn=None,
             interleave_fn=None):
    """512-step GRU scan.

    ag_fn(chunk) is called right after the last y0ex block of `chunk` is
    written, so the pairwise AllGather for that chunk overlaps the scan.
    interleave_fn(u) is called once per step to emit one P-phase tile into
    the PE queue (filling the scan's idle PE slots)."""
    nc_ = nc
    tag = f"s{layer}"
    wpool = ctx.enter_context(tc.tile_pool(name=f"whh_{tag}", bufs=1))
    cpool = ctx.enter_context(tc.tile_pool(name=f"c_{tag}", bufs=1))
    gxp = ctx.enter_context(tc.tile_pool(name=f"gx_{tag}", bufs=2))
    psum = ctx.enter_context(tc.tile_pool(name=f"ps_{tag}", bufs=2, space="PSUM"))
    gp = ctx.enter_context(tc.tile_pool(name=f"g_{tag}", bufs=2))
    yp = ctx.enter_context(tc.tile_pool(name=f"y_{tag}", bufs=2))

    whh = wpool.tile([128, F, G], FP8)
    nc_.sync.dma_start(whh[:], whhT_dram.ap().rearrange("(k p) g -> p k g", p=128))
    ident = cpool.tile([128, 128], FP8)
    nc_.sync.dma_start(ident[:], ident_dram.ap())
    # nbias comes pre-broadcast (and pre-scaled by WSCALE) from the host
    nbx = cpool.tile([128, F, BQ], BF16)
    nc_.sync.dma_start(nbx[:], nbias_dram.ap().rearrange("p (f b) -> p f b", b=BQ))
    zero_bf = cpool.tile([128, F, BQ], BF16)
    nc_.vector.memset(zero_bf[:], 0.0)

    gx_r = gx_dram.ap().rearrange("p (blk m c) -> p blk m c", m=M12, c=TBLK * BQ)
    y1_r = None
    if y1T_dram is not None:
        y1_r = y1T_dram.ap().rearrange("(f p) c -> p f c", p=128)

    def load_block(blk):
        t = gxp.tile([128, M12, TBLK * BQ], BF16)
        nc_.sync.dma_start(t[:], gx_r[:, blk, :, :])
        return t

    def write_block(wb, y1sb):
        if layer == 0:
            chunk = ((S - 1 - wb * TBLK) * BQ) // SBC
            y0e = y0ex_chunks[chunk].ap()
            lo = (S - 1 - wb * TBLK) * BQ - chunk * SBC
            for f in range(F):
                dst = bass.AP(
                    tensor=y0e.tensor,
                    offset=f * 128 * SBC + lo,
                    ap=[[SBC, 128], [-BQ, TBLK], [1, BQ]],
                )
                src = y0own[:, f, ts(wb, TBLK * BQ)].rearrange(
                    "p (t b) -> p t b", b=BQ)
                nc_.sync.dma_start(dst, src)
        else:
            nc_.sync.dma_start(y1_r[:, :, ts(wb, TBLK * BQ)], y1sb[:])

    # scan state
    hm1 = zero_bf[:]
    gxb_cur = load_block(0)
    gxb_next = None
    y1sb_cur = None

    # tile orders inside the PE burst: r,z consume h halves in order so the
    # next step can start as soon as the low half of h lands; the n group is
    # plain fold-major.
    zr_order = [(f, k) for f in range(F) for k in (0, 1)] + \
               [(f, k) for f in range(F) for k in (2, 3)]
    n_order = [(f, k) for f in range(F) for k in range(F)]

    for u in range(S):
        blk, j = divmod(u, TBLK)
        if j == 0:
            if u > 0:
                gxb_cur = gxb_next
            if blk + 1 < NB:
                gxb_next = load_block(blk + 1)
            if layer == 1:
                y1sb_cur = yp.tile([128, F, TBLK * BQ], BF16, tag="y1sb")

        # ---- PE burst, gate group order r, z, n.  Each gate tile is padded
        # to a full 2KB PSUM bank: bank-granular WAR tracking otherwise makes
        # next step's injections wait on this step's slowest PSUM reader. ----
        def ps_tile(tg):
            t = psum.tile([128, 512], F32, tag=tg)
            return t[:, 0: F * BQ].rearrange("p (f b) -> p f b", b=BQ)

        psr = ps_tile("r")
        psz = ps_tile("z")
        psn = ps_tile("n")

        for gate, ps, order, inj, m0 in (
            ("r", psr, zr_order, gxb_cur[:, 0:F, ts(j, BQ)], 0),
            ("z", psz, zr_order, gxb_cur[:, F: 2 * F, ts(j, BQ)], F),
            ("n", psn, n_order, nbx[:], 2 * F),
        ):
            nc_.tensor.matmul(ps[:], lhsT=ident[:], rhs=inj,
                              start=True, stop=False, skip_group_check=True)
            last = order[-1]
            for (f, k) in order:
                nc_.tensor.matmul(ps[:, f, :],
                                  lhsT=whh[:, k, ts(m0 + f, 128)],
                                  rhs=hm1[:, k, :],
                                  start=False, stop=((f, k) == last),
                                  skip_group_check=True)

        # ---- gate math; critical chain: t1 -> t2 -> tanh -> m -> h.
        # ACT queue: sig_r, sig_z, omz, tanh.  DVE: t1, t2, p1, m, h. ----
        r = gp.tile([128, F, BQ], F32, tag="r")
        nc_.scalar.activation(r[:], psr[:], AF.Sigmoid, scale=SINV)
        z = gp.tile([128, F, BQ], F32, tag="z")
        nc_.scalar.activation(z[:], psz[:], AF.Sigmoid, scale=SINV)
        omz = gp.tile([128, F, BQ], F32, tag="omz")
        nc_.scalar.activation(omz[:], psz[:], AF.Sigmoid, scale=-SINV)

        t1 = gp.tile([128, F, BQ], F32, tag="t1")
        nc_.vector.scalar_tensor_tensor(t1[:], psn[:], SINV, r[:],
                                        ALU.mult, ALU.mult)
        t2 = gp.tile([128, F, BQ], F32, tag="t2")
        nc_.vector.tensor_tensor(t2[:], t1[:],
                                 gxb_cur[:, 2 * F: 3 * F, ts(j, BQ)], ALU.add)
        n = gp.tile([128, F, BQ], F32, tag="n")
        nc_.scalar.activation(n[:], t2[:], AF.Tanh)

        p1 = gp.tile([128, F, BQ], F32, tag="p1")
        nc_.vector.tensor_tensor(p1[:], z[:], hm1, ALU.mult)
        m = gp.tile([128, F, BQ], F32, tag="m")
        nc_.vector.tensor_tensor(m[:], omz[:], n[:], ALU.mult)

        if layer == 0:
            hslot = y0own[:, :, ts(u, BQ)]
        else:
            hslot = y1sb_cur[:, :, ts(j, BQ)]
        # h = m + p1, low half first so the next burst can start early
        nc_.vector.tensor_tensor(hslot[:, 0:2, :], m[:, 0:2, :],
                                 p1[:, 0:2, :], ALU.add)
        nc_.vector.tensor_tensor(hslot[:, 2:4, :], m[:, 2:4, :],
                                 p1[:, 2:4, :], ALU.add)
        hm1 = hslot

        if interleave_fn is not None:
            interleave_fn(u)
        if j == TBLK - 1:
            write_block(blk, y1sb_cur)
            if ag_fn is not None and (blk + 1) % NBC == 0:
                ag_fn(((S - 1 - blk * TBLK) * BQ) // SBC)


def build_program(debug=False):
    nc = bacc.Bacc("TRN2", target_bir_lowering=False, debug=debug,
                   num_devices=NCORE)

    def din(name, shape, dt):
        return nc.dram_tensor(name, list(shape), dt, kind="ExternalInput")

    xT = din("xT", (I, SB), BF16)
    wih0T = din("wih0T", (I, G), BF16)
    whh0T = din("whh0T", (H, G), FP8)
    wih1T = din("wih1T", (2 * H, G), BF16)
    whh1T = din("whh1T", (H, G), FP8)
    gbias0 = din("gbias0", (128, M12), F32)
    gbias1 = din("gbias1", (128, M12), F32)
    nbias0 = din("nbias0", (128, F * BQ), BF16)
    nbias1 = din("nbias1", (128, F * BQ), BF16)
    ident = din("ident", (128, 128), FP8)

    y1T = nc.dram_tensor("y1T", [H, SB], BF16, kind="ExternalOutput")

    gx0T = nc.dram_tensor("gx0T", [128, NB * M12 * TBLK * BQ], BF16)
    gx1T = nc.dram_tensor("gx1T", [128, NB * M12 * TBLK * BQ], BF16)
    y0ex_chunks = [nc.dram_tensor(f"y0ex{c}", [H, SBC], BF16)
                   for c in range(NCH)]
    y0g_chunks = [nc.dram_tensor(f"y0g{c}", [2, H, SBC], BF16)
                  for c in range(NCH)]
    y0loc_chunks = [nc.dram_tensor(f"y0loc{c}", [H, SBC], BF16)
                    for c in range(NCH)]

    groups = [[2 * q, 2 * q + 1] for q in range(4)]

    with tile.TileContext(nc) as tc:
        with ExitStack() as ctx:
            # ---- P0 stream: layer-0 input projection, head chunks up front,
            # the rest interleaved one tile per S0 step ----
            xpool = ctx.enter_context(tc.tile_pool(name="xsb", bufs=1))
            xsb = xpool.tile([128, KI0, SB], BF16)
            nc.sync.dma_start(xsb[:], xT.ap().rearrange("(k p) c -> p k c", p=128))
            y0pool = ctx.enter_context(tc.tile_pool(name="y0own", bufs=1))
            y0own = y0pool.tile([128, F, SB], BF16)
            with ExitStack() as p0ctx:
                p0 = PStream(p0ctx, tc, nc, wih0T, gbias0, gx0T, KI0,
                             lambda c: [xsb[:, k, ts(c, NCOL)] for k in range(KI0)],
                             "p0", psum_bufs=2)
                p0.emit_chunks(2)

                # ---- S0 scan (+ interleaved P0 tiles); y0own holds the h
                # sequence in SBUF.  Pairwise AllGathers fire per chunk. ----
                rank = nc.gpsimd.cc_rank(groups)

                def ag_fn(c):
                    nc.gpsimd.collective_compute(
                        "AllGather", ALU.bypass,
                        ins=[y0ex_chunks[c].ap()], outs=[y0g_chunks[c].ap()],
                        replica_groups=groups,
                    )
                    with tc.If(rank < 1) as cmp:
                        for rr in range(4):
                            nc.gpsimd.dma_start(
                                y0loc_chunks[c].ap()[ts(rr, 128), :],
                                y0g_chunks[c].ap()[1, ts(rr, 128), :])
                    with cmp.Else():
                        for rr in range(4):
                            nc.gpsimd.dma_start(
                                y0loc_chunks[c].ap()[ts(rr, 128), :],
                                y0g_chunks[c].ap()[0, ts(rr, 128), :])

                with ExitStack() as sctx:
                    _s_phase(sctx, tc, nc, whh0T, nbias0, gx0T, 0, y0own, None,
                             ident, y0ex_chunks, ag_fn=ag_fn,
                             interleave_fn=lambda u: p0.emit_tile())
                p0.finish()

            # ---- P1 stream: head chunks serial (waits on AG chunk 0), the
            # rest interleaved one tile per S1 step ----
            with ExitStack() as p1ctx:
                ppool = p1ctx.enter_context(tc.tile_pool(name="part", bufs=3))
                y0l_r = [t.ap().rearrange("(k p) c -> p k c", p=128)
                         for t in y0loc_chunks]
                cpc = SBC // NCOL  # NCOL chunks per AG chunk

                def rhs1(c):
                    part = ppool.tile([128, F, NCOL], BF16)
                    ch, off = divmod(c, cpc)
                    nc.sync.dma_start(part[:], y0l_r[ch][:, :, ts(off, NCOL)])
                    return [y0own[:, k, ts(c, NCOL)] for k in range(F)] + \
                           [part[:, k, :] for k in range(F)]

                p1 = PStream(p1ctx, tc, nc, wih1T, gbias1, gx1T, KI1, rhs1,
                             "p1", psum_bufs=2)
                p1.emit_chunks(2)

                # ---- S1: layer-1 scan (+ interleaved P1 tiles) -> y1T ----
                with ExitStack() as sctx:
                    _s_phase(sctx, tc, nc, whh1T, nbias1, gx1T, 1, None, y1T,
                             ident, None,
                             interleave_fn=lambda u: p1.emit_tile())
                p1.finish()

    nc.compile()
    return nc


_PROGRAM_CACHE = {}


def _get_program():
    if "nc" not in _PROGRAM_CACHE:
        _PROGRAM_CACHE["nc"] = build_program()
    return _PROGRAM_CACHE["nc"]


def _host_inputs(inputs):
    """Build the 8 per-core input maps from the full problem inputs."""
    bf = ml_dtypes.bfloat16
    f8 = ml_dtypes.float8_e4m3
    x = np.asarray(inputs["input"], np.float32)            # (S, B, I)
    in_maps = []
    for c in range(NCORE):
        fwd = c % 2 == 0
        q = c // 2
        d = "f" if fwd else "b"
        xq = x[:, q * BQ:(q + 1) * BQ, :]
        if not fwd:
            xq = xq[::-1]
        xTv = np.ascontiguousarray(xq.transpose(2, 0, 1).reshape(I, SB))

        def wT(wname):
            return np.ascontiguousarray(np.asarray(inputs[wname], np.float32).T)

        wih0 = wT(f"Wih_{d}0")        # (I, G)
        whh0 = wT(f"Whh_{d}0")        # (H, G)
        wih1_full = wT(f"Wih_{d}1")   # (2H, G); rows = y0 features [hf | hb]
        own_sl = slice(0, H) if fwd else slice(H, 2 * H)
        par_sl = slice(H, 2 * H) if fwd else slice(0, H)
        wih1 = np.concatenate([wih1_full[own_sl], wih1_full[par_sl]], axis=0)
        whh1 = wT(f"Whh_{d}1")

        def gbias(layer):
            bih = np.asarray(inputs[f"bih_{d}{layer}"], np.float32)
            bhh = np.asarray(inputs[f"bhh_{d}{layer}"], np.float32)
            gb = np.concatenate([bih[:2 * H] + bhh[:2 * H], bih[2 * H:]])
            gb = np.ascontiguousarray(gb.reshape(M12, 128).T)  # [128, M12]
            gb[:, : 2 * F] *= WSCALE   # r,z chunks pre-scaled (see _p_phase)
            return gb

        def nbias(layer):
            bhh = np.asarray(inputs[f"bhh_{d}{layer}"], np.float32)
            nb = (bhh[2 * H:] * WSCALE).reshape(F, 128).T  # [128, F], scaled
            return np.ascontiguousarray(
                np.broadcast_to(nb[:, :, None], (128, F, BQ)).reshape(
                    128, F * BQ)).astype(bf)

        in_maps.append({
            "xT": xTv.astype(bf),
            "wih0T": wih0.astype(bf),
            "whh0T": (whh0 * WSCALE).astype(f8),
            "wih1T": wih1.astype(bf),
            "whh1T": (whh1 * WSCALE).astype(f8),
            "gbias0": gbias(0), "gbias1": gbias(1),
            "nbias0": nbias(0), "nbias1": nbias(1),
            "ident": np.eye(128).astype(f8),
        })
    return in_maps


def kernel(**inputs) -> np.ndarray:
    nc = _get_program()
    in_maps = _host_inputs(inputs)
    trace = bool(int(os.environ.get("BIGRU_TRACE", "0")))
    kw = {}
    if trace and os.environ.get("BIGRU_TRACE_DIR"):
        kw["tmpdir"] = os.environ["BIGRU_TRACE_DIR"]
    res = run_bass_kernel_spmd(nc, in_maps, list(range(NCORE)), trace=trace, **kw)
    if trace and res.exec_time_ns is not None:
        print(f"HW exec time: {res.exec_time_ns} ns")
        _PROGRAM_CACHE["exec_time_ns"] = res.exec_time_ns
        _PROGRAM_CACHE["profile_json"] = res.profile_json

    out = np.empty((S, B, 2 * H), np.float32)
    for c in range(NCORE):
        fwd = c % 2 == 0
        q = c // 2
        y = np.asarray(res.results[c]["y1T"], dtype=np.float32)
        y = y.reshape(H, S, BQ).transpose(1, 2, 0)  # (S, BQ, H)
        if not fwd:
            y = y[::-1]
        out[:, q * BQ:(q + 1) * BQ, (0 if fwd else H):(H if fwd else 2 * H)] = y
    return out


# revision 34
# speedup vs baseline: 1.0824x; 1.0824x over previous
"""BiGRU (S=512, B=64, I=256, H=512, L=2) Trainium2 Bass kernel.

Strategy: 4-way batch split x 2-way direction split across 8 NeuronCores.
Cores 0-3 run the forward GRU chain (layers 0 and 1) for batch quarters
0-3; cores 4-7 run the backward chain (fed time-reversed input, so the
device program is identical on every core).  Per layer each core does:

  P-phase: gxT = Wih @ xT + bias  (big efficient matmul, bf16, weights
           stationary, all timesteps as the moving operand), written to
           DRAM in a scan-friendly blocked layout.
  S-phase: 512-step sequential GRU scan.  Per step the 48 Whh weight
           tiles stream through the PE; the moving operand is the PAIR
           (m, p1) with h = m + p1 (linearity of the recurrent matmul),
           so the final h combine is off the critical path (done lazily
           on GpSimd one step later).  Gate math chain:
             z  = sigmoid(ghz)         omz = sigmoid(-ghz)
             p1 = z * h_prev           r   = sigmoid(ghr)
             t  = tanh(r*ghn + gxn)  (split in two fold-halves)
             m  = omz * t
           The n-gate PSUM is split in two half tiles so the tanh chain
           starts as soon as the first half's matmuls land.

Between layers the forward/backward partners exchange their hidden-state
sequences with pairwise AllGathers split into 4 time-chunks, each fired
as soon as its chunk of y0ex is written during the scan (overlapping the
collective with the scan).  Final un-transpose / un-reverse of the
output happens on the host.
"""

import os
import sys
import numpy as np

for _p in ("/opt/trn_rl_repo", "/root/.axon_site/_ro/trn_rl_repo"):
    if os.path.isdir(_p) and _p not in sys.path:
        sys.path.insert(0, _p)

import ml_dtypes
from contextlib import ExitStack

import concourse.bass as bass
import concourse.tile as tile
from concourse import bacc, mybir
from concourse.bass import ts
from concourse.bass_utils import run_bass_kernel_spmd

BF16 = mybir.dt.bfloat16
FP8 = mybir.dt.float8e4
F32 = mybir.dt.float32
AF = mybir.ActivationFunctionType
ALU = mybir.AluOpType

# Whh is stored in fp8-e4m3 scaled so max|W| -> 240; the descale folds into
# the activation `scale` operand (gates) / one fused scalar_tensor_tensor (n).
WSCALE = float(240.0 * np.sqrt(512.0))
SINV = float(1.0 / WSCALE)

S, B, I, H, L = 512, 64, 256, 512, 2
G = 3 * H            # 1536 gate rows (r, z, n)
NCORE = 8
BQ = B // 4          # 16 batch per core
SB = S * BQ          # 8192 moving columns
F = H // 128         # 4 h-fold chunks
M12 = G // 128       # 12 gate chunks
KI0 = I // 128       # 2 contraction chunks, layer-0 input proj
KI1 = 2 * H // 128   # 8 contraction chunks, layer-1 input proj
NCOL = 512           # P-phase moving chunk width
TBLK = 16            # gx prefetch / y writeback block (steps)
NB = S // TBLK       # 32 blocks
NCH = 8              # AllGather chunks
NBC = NB // NCH      # blocks per chunk
SBC = SB // NCH      # columns per chunk
UB = NCOL // BQ      # steps covered by one P-phase column chunk (32)


class PStream:
    """Input-projection tile stream: gx = W @ xT + bias, bf16, written to
    gx_dram in blocked layout [128, NB, M12, TBLK*BQ].  Tiles can be emitted
    in bulk (emit_chunks/finish) or one at a time (emit_tile) so they
    interleave into the scan's idle PE slots."""

    def __init__(self, ctx, tc, nc, wT_dram, gbias_dram, gx_dram, ki, rhs_fn,
                 tag, psum_bufs=4):
        self.nc = nc
        self.ki = ki
        self.rhs_fn = rhs_fn
        self.wpool = ctx.enter_context(tc.tile_pool(name=f"w_{tag}", bufs=1))
        self.bpool = ctx.enter_context(tc.tile_pool(name=f"b_{tag}", bufs=1))
        self.psum = ctx.enter_context(
            tc.tile_pool(name=f"ps_{tag}", bufs=psum_bufs, space="PSUM"))
        self.stg = ctx.enter_context(tc.tile_pool(name=f"st_{tag}", bufs=4))

        self.wsb = self.wpool.tile([128, ki, G], BF16)
        nc.sync.dma_start(self.wsb[:],
                          wT_dram.ap().rearrange("(k p) g -> p k g", p=128))
        self.gb = self.bpool.tile([128, M12], F32)
        nc.sync.dma_start(self.gb[:], gbias_dram.ap())

        self.gx_r = gx_dram.ap().rearrange("p (blk m c) -> p blk m c",
                                           m=M12, c=TBLK * BQ)
        self.nub = NCOL // (TBLK * BQ)  # u-blocks per column chunk (2)
        self.tiles = [(c, m) for c in range(SB // NCOL) for m in range(M12)]
        self.pos = 0
        self.rhs_tiles = None

    def emit_tile(self):
        if self.pos >= len(self.tiles):
            return False
        nc_ = self.nc
        c, m = self.tiles[self.pos]
        self.pos += 1
        if m == 0:
            self.rhs_tiles = self.rhs_fn(c)  # list of ki [128, NCOL] bf16 APs
        ps = self.psum.tile([128, NCOL], F32)
        for k in range(self.ki):
            nc_.tensor.matmul(
                ps[:],
                lhsT=self.wsb[:, k, ts(m, 128)],
                rhs=self.rhs_tiles[k],
                start=(k == 0),
                stop=(k == self.ki - 1),
            )
        # r,z chunks (m<8) are pre-scaled by WSCALE so the S-phase can
        # descale the whole PSUM (Whh fp8 part + injected gx) at once.
        # gbias for m<8 comes pre-scaled from the host.
        out = self.stg.tile([128, NCOL], BF16)
        sc = WSCALE if m < 2 * F else 1.0
        if m % 2 == 0:
            nc_.scalar.activation(out[:], ps[:], AF.Identity,
                                  bias=self.gb[:, m: m + 1], scale=sc)
        else:
            nc_.vector.tensor_scalar(out[:], ps[:], sc, self.gb[:, m: m + 1],
                                     ALU.mult, ALU.add)
        nc_.sync.dma_start(
            self.gx_r[:, ts(c, self.nub), m, :],
            out[:].rearrange("p (i c) -> p i c", c=TBLK * BQ),
        )
        return True

    def emit_chunks(self, n):
        for _ in range(n * M12):
            self.emit_tile()

    def finish(self):
        while self.emit_tile():
            pass


def _s_phase(ctx, tc, nc, whhT_dram, nbias_dram, gx_dram, layer, y0own,
             y1T_dram, ident_dram, y0ex_chunks, ag_fn=None,
             interleave_fn=None):
    """512-step GRU scan.

    ag_fn(chunk) is called right after the last y0ex block of `chunk` is
    written, so the pairwise AllGather for that chunk overlaps the scan.
    interleave_fn(u) is called once per step to emit one P-phase tile into
    the PE queue (filling the scan's idle PE slots)."""
    nc_ = nc
    tag = f"s{layer}"
    wpool = ctx.enter_context(tc.tile_pool(name=f"whh_{tag}", bufs=1))
    cpool = ctx.enter_context(tc.tile_pool(name=f"c_{tag}", bufs=1))
    gxp = ctx.enter_context(tc.tile_pool(name=f"gx_{tag}", bufs=2))
    psum = ctx.enter_context(tc.tile_pool(name=f"ps_{tag}", bufs=1, space="PSUM"))
    gp = ctx.enter_context(tc.tile_pool(name=f"g_{tag}", bufs=2))
    yp = ctx.enter_context(tc.tile_pool(name=f"y_{tag}", bufs=2))

    whh = wpool.tile([128, F, G], FP8)
    nc_.sync.dma_start(whh[:], whhT_dram.ap().rearrange("(k p) g -> p k g", p=128))
    ident = cpool.tile([128, 128], FP8)
    nc_.sync.dma_start(ident[:], ident_dram.ap())
    # nbias comes pre-broadcast (and pre-scaled by WSCALE) from the host
    nbx = cpool.tile([128, F, BQ], BF16)
    nc_.sync.dma_start(nbx[:], nbias_dram.ap().rearrange("p (f b) -> p f b", b=BQ))
    zero_bf = cpool.tile([128, F, BQ], BF16)
    nc_.vector.memset(zero_bf[:], 0.0)

    gx_r = gx_dram.ap().rearrange("p (blk m c) -> p blk m c", m=M12, c=TBLK * BQ)
    y1_r = None
    if y1T_dram is not None:
        y1_r = y1T_dram.ap().rearrange("(f p) c -> p f c", p=128)

    def load_block(blk):
        t = gxp.tile([128, M12, TBLK * BQ], BF16)
        nc_.sync.dma_start(t[:], gx_r[:, blk, :, :])
        return t

    def write_block(wb, y1sb):
        if layer == 0:
            chunk = ((S - 1 - wb * TBLK) * BQ) // SBC
            y0e = y0ex_chunks[chunk].ap()
            lo = (S - 1 - wb * TBLK) * BQ - chunk * SBC
            for f in range(F):
                dst = bass.AP(
                    tensor=y0e.tensor,
                    offset=f * 128 * SBC + lo,
                    ap=[[SBC, 128], [-BQ, TBLK], [1, BQ]],
                )
                src = y0own[:, f, ts(wb, TBLK * BQ)].rearrange(
                    "p (t b) -> p t b", b=BQ)
                nc_.sync.dma_start(dst, src)
        else:
            nc_.sync.dma_start(y1_r[:, :, ts(wb, TBLK * BQ)], y1sb[:])

    # scan state
    hm1 = zero_bf[:]
    gxb_cur = load_block(0)
    gxb_next = None
    y1sb_cur = None

    # tile orders inside the PE burst: r,z consume h halves in order so the
    # next step can start as soon as the low half of h lands; the n group is
    # plain fold-major.
    zr_order = [(f, k) for f in range(F) for k in (0, 1)] + \
               [(f, k) for f in range(F) for k in (2, 3)]
    n_half = [[(f, k) for f in (0, 1) for k in range(F)],
              [(f - 2, k) for f in (2, 3) for k in range(F)]]

    for u in range(S):
        blk, j = divmod(u, TBLK)
        if j == 0:
            if u > 0:
                gxb_cur = gxb_next
            if blk + 1 < NB:
                gxb_next = load_block(blk + 1)
            if layer == 1:
                y1sb_cur = yp.tile([128, F, TBLK * BQ], BF16, tag="y1sb")

        # ---- PE burst, gate group order r, z, n0, n1.  Gate tiles are
        # padded to full 2KB PSUM banks.  The n group is split in two fold
        # halves so the tanh chain starts while the n1 matmuls still run. ----
        def ps_tile(tg, nf):
            t = psum.tile([128, 512], F32, tag=tg)
            return t[:, 0: nf * BQ].rearrange("p (f b) -> p f b", b=BQ)

        psr = ps_tile("r", F)
        psz = ps_tile("z", F)
        psn0 = ps_tile("n0", 2)
        psn1 = ps_tile("n1", 2)

        for gate, ps, order, inj, m0 in (
            ("r", psr, zr_order, gxb_cur[:, 0:F, ts(j, BQ)], 0),
            ("z", psz, zr_order, gxb_cur[:, F: 2 * F, ts(j, BQ)], F),
            ("n0", psn0, n_half[0], nbx[:, 0:2, :], 2 * F),
            ("n1", psn1, n_half[1], nbx[:, 2:4, :], 2 * F + 2),
        ):
            nc_.tensor.matmul(ps[:], lhsT=ident[:], rhs=inj,
                              start=True, stop=False, skip_group_check=True)
            last = order[-1]
            for (f, k) in order:
                nc_.tensor.matmul(ps[:, f, :],
                                  lhsT=whh[:, k, ts(m0 + f, 128)],
                                  rhs=hm1[:, k, :],
                                  start=False, stop=((f, k) == last),
                                  skip_group_check=True)

        # ---- gate math; critical chain per half: t1 -> t2 -> tanh -> m -> h.
        # ACT queue: sig_r, sig_z, omz, tanh0, tanh1.
        # DVE queue: t1h0, t2h0, t1h1, t2h1, p1, m0, h0, m1, h1. ----
        r = gp.tile([128, F, BQ], F32, tag="r")
        nc_.scalar.activation(r[:], psr[:], AF.Sigmoid, scale=SINV)
        z = gp.tile([128, F, BQ], F32, tag="z")
        nc_.scalar.activation(z[:], psz[:], AF.Sigmoid, scale=SINV)
        omz = gp.tile([128, F, BQ], F32, tag="omz")
        nc_.scalar.activation(omz[:], psz[:], AF.Sigmoid, scale=-SINV)

        t2 = gp.tile([128, F, BQ], F32, tag="t2")
        n = gp.tile([128, F, BQ], F32, tag="n")
        for hh, psn in ((0, psn0), (1, psn1)):
            sl = ts(hh, 2)
            t1 = gp.tile([128, 2, BQ], F32, tag=f"t1{hh}")
            nc_.vector.scalar_tensor_tensor(t1[:], psn[:], SINV, r[:, sl, :],
                                            ALU.mult, ALU.mult)
            nc_.vector.tensor_tensor(
                t2[:, sl, :], t1[:],
                gxb_cur[:, 2 * F + 2 * hh: 2 * F + 2 * hh + 2, ts(j, BQ)],
                ALU.add)
            nc_.scalar.activation(n[:, sl, :], t2[:, sl, :], AF.Tanh)

        if layer == 0:
            hslot = y0own[:, :, ts(u, BQ)]
        else:
            hslot = y1sb_cur[:, :, ts(j, BQ)]
        p1 = gp.tile([128, F, BQ], F32, tag="p1")
        nc_.vector.tensor_tensor(p1[:], z[:], hm1, ALU.mult)
        m = gp.tile([128, F, BQ], F32, tag="m")
        for hh in (0, 1):
            sl = ts(hh, 2)
            nc_.vector.tensor_tensor(m[:, sl, :], omz[:, sl, :], n[:, sl, :],
                                     ALU.mult)
            nc_.vector.tensor_tensor(hslot[:, sl, :], m[:, sl, :],
                                     p1[:, sl, :], ALU.add)
        hm1 = hslot

        if interleave_fn is not None:
            interleave_fn(u)
        if j == TBLK - 1:
            write_block(blk, y1sb_cur)
            if ag_fn is not None and (blk + 1) % NBC == 0:
                ag_fn(((S - 1 - blk * TBLK) * BQ) // SBC)


def build_program(debug=False):
    nc = bacc.Bacc("TRN2", target_bir_lowering=False, debug=debug,
                   num_devices=NCORE)

    def din(name, shape, dt):
        return nc.dram_tensor(name, list(shape), dt, kind="ExternalInput")

    xT = din("xT", (I, SB), BF16)
    wih0T = din("wih0T", (I, G), BF16)
    whh0T = din("whh0T", (H, G), FP8)
    wih1T = din("wih1T", (2 * H, G), BF16)
    whh1T = din("whh1T", (H, G), FP8)
    gbias0 = din("gbias0", (128, M12), F32)
    gbias1 = din("gbias1", (128, M12), F32)
    nbias0 = din("nbias0", (128, F * BQ), BF16)
    nbias1 = din("nbias1", (128, F * BQ), BF16)
    ident = din("ident", (128, 128), FP8)

    y1T = nc.dram_tensor("y1T", [H, SB], BF16, kind="ExternalOutput")

    gx0T = nc.dram_tensor("gx0T", [128, NB * M12 * TBLK * BQ], BF16)
    gx1T = nc.dram_tensor("gx1T", [128, NB * M12 * TBLK * BQ], BF16)
    y0ex_chunks = [nc.dram_tensor(f"y0ex{c}", [H, SBC], BF16)
                   for c in range(NCH)]
    y0g_chunks = [nc.dram_tensor(f"y0g{c}", [2, H, SBC], BF16)
                  for c in range(NCH)]
    y0loc_chunks = [nc.dram_tensor(f"y0loc{c}", [H, SBC], BF16)
                    for c in range(NCH)]

    groups = [[2 * q, 2 * q + 1] for q in range(4)]

    with tile.TileContext(nc) as tc:
        with ExitStack() as ctx:
            # ---- P0 stream: layer-0 input projection, head chunks up front,
            # the rest interleaved one tile per S0 step ----
            xpool = ctx.enter_context(tc.tile_pool(name="xsb", bufs=1))
            xsb = xpool.tile([128, KI0, SB], BF16)
            nc.sync.dma_start(xsb[:], xT.ap().rearrange("(k p) c -> p k c", p=128))
            y0pool = ctx.enter_context(tc.tile_pool(name="y0own", bufs=1))
            y0own = y0pool.tile([128, F, SB], BF16)
            with ExitStack() as p0ctx:
                p0 = PStream(p0ctx, tc, nc, wih0T, gbias0, gx0T, KI0,
                             lambda c: [xsb[:, k, ts(c, NCOL)] for k in range(KI0)],
                             "p0", psum_bufs=2)
                p0.emit_chunks(2)

                # ---- S0 scan (+ interleaved P0 tiles); y0own holds the h
                # sequence in SBUF.  Pairwise AllGathers fire per chunk. ----
                rank = nc.gpsimd.cc_rank(groups)

                def ag_fn(c):
                    nc.gpsimd.collective_compute(
                        "AllGather", ALU.bypass,
                        ins=[y0ex_chunks[c].ap()], outs=[y0g_chunks[c].ap()],
                        replica_groups=groups,
                    )
                    with tc.If(rank < 1) as cmp:
                        for rr in range(4):
                            nc.gpsimd.dma_start(
                                y0loc_chunks[c].ap()[ts(rr, 128), :],
                                y0g_chunks[c].ap()[1, ts(rr, 128), :])
                    with cmp.Else():
                        for rr in range(4):
                            nc.gpsimd.dma_start(
                                y0loc_chunks[c].ap()[ts(rr, 128), :],
                                y0g_chunks[c].ap()[0, ts(rr, 128), :])

                with ExitStack() as sctx:
                    _s_phase(sctx, tc, nc, whh0T, nbias0, gx0T, 0, y0own, None,
                             ident, y0ex_chunks, ag_fn=ag_fn,
                             interleave_fn=lambda u: p0.emit_tile())
                p0.finish()

            # ---- P1 stream: head chunks serial (waits on AG chunk 0), the
            # rest interleaved one tile per S1 step ----
            with ExitStack() as p1ctx:
                ppool = p1ctx.enter_context(tc.tile_pool(name="part", bufs=3))
                y0l_r = [t.ap().rearrange("(k p) c -> p k c", p=128)
                         for t in y0loc_chunks]
                cpc = SBC // NCOL  # NCOL chunks per AG chunk

                def rhs1(c):
                    part = ppool.tile([128, F, NCOL], BF16)
                    ch, off = divmod(c, cpc)
                    nc.sync.dma_start(part[:], y0l_r[ch][:, :, ts(off, NCOL)])
                    return [y0own[:, k, ts(c, NCOL)] for k in range(F)] + \
                           [part[:, k, :] for k in range(F)]

                p1 = PStream(p1ctx, tc, nc, wih1T, gbias1, gx1T, KI1, rhs1,
                             "p1", psum_bufs=2)
                p1.emit_chunks(2)

                # ---- S1: layer-1 scan (+ interleaved P1 tiles) -> y1T ----
                with ExitStack() as sctx:
                    _s_phase(sctx, tc, nc, whh1T, nbias1, gx1T, 1, None, y1T,
                             ident, None,
                             interleave_fn=lambda u: p1.emit_tile())
                p1.finish()

    nc.compile()
    return nc


_PROGRAM_CACHE = {}


def _get_program():
    if "nc" not in _PROGRAM_CACHE:
        _PROGRAM_CACHE["nc"] = build_program()
    return _PROGRAM_CACHE["nc"]


def _host_inputs(inputs):
    """Build the 8 per-core input maps from the full problem inputs."""
    bf = ml_dtypes.bfloat16
    f8 = ml_dtypes.float8_e4m3
    x = np.asarray(inputs["input"], np.float32)            # (S, B, I)
    in_maps = []
    for c in range(NCORE):
        fwd = c % 2 == 0
        q = c // 2
        d = "f" if fwd else "b"
        xq = x[:, q * BQ:(q + 1) * BQ, :]
        if not fwd:
            xq = xq[::-1]
        xTv = np.ascontiguousarray(xq.transpose(2, 0, 1).reshape(I, SB))

        def wT(wname):
            return np.ascontiguousarray(np.asarray(inputs[wname], np.float32).T)

        wih0 = wT(f"Wih_{d}0")        # (I, G)
        whh0 = wT(f"Whh_{d}0")        # (H, G)
        wih1_full = wT(f"Wih_{d}1")   # (2H, G); rows = y0 features [hf | hb]
        own_sl = slice(0, H) if fwd else slice(H, 2 * H)
        par_sl = slice(H, 2 * H) if fwd else slice(0, H)
        wih1 = np.concatenate([wih1_full[own_sl], wih1_full[par_sl]], axis=0)
        whh1 = wT(f"Whh_{d}1")

        def gbias(layer):
            bih = np.asarray(inputs[f"bih_{d}{layer}"], np.float32)
            bhh = np.asarray(inputs[f"bhh_{d}{layer}"], np.float32)
            gb = np.concatenate([bih[:2 * H] + bhh[:2 * H], bih[2 * H:]])
            gb = np.ascontiguousarray(gb.reshape(M12, 128).T)  # [128, M12]
            gb[:, : 2 * F] *= WSCALE   # r,z chunks pre-scaled (see _p_phase)
            return gb

        def nbias(layer):
            bhh = np.asarray(inputs[f"bhh_{d}{layer}"], np.float32)
            nb = (bhh[2 * H:] * WSCALE).reshape(F, 128).T  # [128, F], scaled
            return np.ascontiguousarray(
                np.broadcast_to(nb[:, :, None], (128, F, BQ)).reshape(
                    128, F * BQ)).astype(bf)

        in_maps.append({
            "xT": xTv.astype(bf),
            "wih0T": wih0.astype(bf),
            "whh0T": (whh0 * WSCALE).astype(f8),
            "wih1T": wih1.astype(bf),
            "whh1T": (whh1 * WSCALE).astype(f8),
            "gbias0": gbias(0), "gbias1": gbias(1),
            "nbias0": nbias(0), "nbias1": nbias(1),
            "ident": np.eye(128).astype(f8),
        })
    return in_maps


def kernel(**inputs) -> np.ndarray:
    nc = _get_program()
    in_maps = _host_inputs(inputs)
    trace = bool(int(os.environ.get("BIGRU_TRACE", "0")))
    kw = {}
    if trace and os.environ.get("BIGRU_TRACE_DIR"):
        kw["tmpdir"] = os.environ["BIGRU_TRACE_DIR"]
    res = run_bass_kernel_spmd(nc, in_maps, list(range(NCORE)), trace=trace, **kw)
    if trace and res.exec_time_ns is not None:
        print(f"HW exec time: {res.exec_time_ns} ns")
        _PROGRAM_CACHE["exec_time_ns"] = res.exec_time_ns
        _PROGRAM_CACHE["profile_json"] = res.profile_json

    out = np.empty((S, B, 2 * H), np.float32)
    for c in range(NCORE):
        fwd = c % 2 == 0
        q = c // 2
        y = np.asarray(res.results[c]["y1T"], dtype=np.float32)
        y = y.reshape(H, S, BQ).transpose(1, 2, 0)  # (S, BQ, H)
        if not fwd:
            y = y[::-1]
        out[:, q * BQ:(q + 1) * BQ, (0 if fwd else H):(H if fwd else 2 * H)] = y
    return out


# revision 44
# speedup vs baseline: 1.0848x; 1.0021x over previous
"""BiGRU (S=512, B=64, I=256, H=512, L=2) Trainium2 Bass kernel.

Strategy: 4-way batch split x 2-way direction split across 8 NeuronCores.
Cores 0-3 run the forward GRU chain (layers 0 and 1) for batch quarters
0-3; cores 4-7 run the backward chain (fed time-reversed input, so the
device program is identical on every core).  Per layer each core does:

  P-stream: gxT = Wih @ xT + bias (bf16 weights stationary, N=512 moving
           chunks), written to DRAM in a scan-blocked layout
           [128, NB, M12, TBLK*BQ].  Only a 2-chunk head runs up front;
           the remaining tiles are interleaved ONE PER SCAN STEP into the
           scan's idle PE slots, so the projection costs ~no wall time.
  S-phase: 512-step sequential GRU scan.  Whh is fp8-e4m3 (globally
           scaled; descale folded into activation `scale` operands), so
           the 48 LDWEIGHTS+MATMUL pairs per step run at ~20ns each.
           gx arrives via 16-step blocked prefetch (large DMA descriptors).
           Gate PSUM groups r, z, n0, n1 (n in fold-halves so the tanh
           chain starts while the n1 matmuls still run):
             r = sig(SINV*psr)  z = sig(SINV*psz)  omz = sig(-SINV*psz)
             per half: t = tanh((SINV*psn)*r + gxn);  m = omz*t
             h = m + z*h_prev   (low half first: next burst starts on it;
           the z/r matmul k-order consumes h low-half before high-half)

Between layers the forward/backward partners exchange their hidden-state
sequences with pairwise AllGathers split into 8 time-chunks, each fired
as soon as its chunk of y0ex is written during the scan (overlapping the
collective with the scan).  Final un-transpose / un-reverse of the
output happens on the host.
"""

import os
import sys
import numpy as np

for _p in ("/opt/trn_rl_repo", "/root/.axon_site/_ro/trn_rl_repo"):
    if os.path.isdir(_p) and _p not in sys.path:
        sys.path.insert(0, _p)

import ml_dtypes
from contextlib import ExitStack

import concourse.bass as bass
import concourse.tile as tile
from concourse import bacc, mybir
from concourse.bass import ts
from concourse.bass_utils import run_bass_kernel_spmd

BF16 = mybir.dt.bfloat16
FP8 = mybir.dt.float8e4
F32 = mybir.dt.float32
AF = mybir.ActivationFunctionType
ALU = mybir.AluOpType

# Whh is stored in fp8-e4m3 scaled so max|W| -> 240; the descale folds into
# the activation `scale` operand (gates) / one fused scalar_tensor_tensor (n).
WSCALE = float(240.0 * np.sqrt(512.0))
SINV = float(1.0 / WSCALE)

S, B, I, H, L = 512, 64, 256, 512, 2
G = 3 * H            # 1536 gate rows (r, z, n)
NCORE = 8
BQ = B // 4          # 16 batch per core
SB = S * BQ          # 8192 moving columns
F = H // 128         # 4 h-fold chunks
M12 = G // 128       # 12 gate chunks
KI0 = I // 128       # 2 contraction chunks, layer-0 input proj
KI1 = 2 * H // 128   # 8 contraction chunks, layer-1 input proj
NCOL = 512           # P-phase moving chunk width
TBLK = 16            # gx prefetch / y writeback block (steps)
NB = S // TBLK       # 32 blocks
NCH = 8              # AllGather chunks
NBC = NB // NCH      # blocks per chunk
SBC = SB // NCH      # columns per chunk
UB = NCOL // BQ      # steps covered by one P-phase column chunk (32)


class PStream:
    """Input-projection tile stream: gx = W @ xT + bias, bf16, written to
    gx_dram in blocked layout [128, NB, M12, TBLK*BQ].  Tiles can be emitted
    in bulk (emit_chunks/finish) or one at a time (emit_tile) so they
    interleave into the scan's idle PE slots."""

    def __init__(self, ctx, tc, nc, wT_dram, gbias_dram, gx_dram, ki, rhs_fn,
                 tag, psum_bufs=4, split=1):
        self.nc = nc
        self.ki = ki
        self.rhs_fn = rhs_fn
        self.split = split   # emit_tile() calls per tile (k-loop halves)
        self.phase = 0
        self.cur_ps = None
        self.wpool = ctx.enter_context(tc.tile_pool(name=f"w_{tag}", bufs=1))
        self.bpool = ctx.enter_context(tc.tile_pool(name=f"b_{tag}", bufs=1))
        self.psum = ctx.enter_context(
            tc.tile_pool(name=f"ps_{tag}", bufs=psum_bufs, space="PSUM"))
        self.stg = ctx.enter_context(tc.tile_pool(name=f"st_{tag}", bufs=4))

        self.wsb = self.wpool.tile([128, ki, G], BF16)
        nc.sync.dma_start(self.wsb[:],
                          wT_dram.ap().rearrange("(k p) g -> p k g", p=128))
        self.gb = self.bpool.tile([128, M12], F32)
        nc.sync.dma_start(self.gb[:], gbias_dram.ap())

        self.gx_r = gx_dram.ap().rearrange("p (blk m c) -> p blk m c",
                                           m=M12, c=TBLK * BQ)
        self.nub = NCOL // (TBLK * BQ)  # u-blocks per column chunk (2)
        self.tiles = [(c, m) for c in range(SB // NCOL) for m in range(M12)]
        self.pos = 0
        self.rhs_tiles = None

    def emit_tile(self):
        """Emit 1/split of one projection tile's matmuls (one call per scan
        step keeps the inserted PE work within the scan's idle slot)."""
        if self.pos >= len(self.tiles):
            return False
        nc_ = self.nc
        c, m = self.tiles[self.pos]
        k0 = self.phase * self.ki // self.split
        k1 = (self.phase + 1) * self.ki // self.split
        if k0 == 0:
            if m == 0:
                self.rhs_tiles = self.rhs_fn(c)  # ki [128, NCOL] bf16 APs
            pnew = self.psum.tile([128, NCOL], F32)
            self.cur_ps = pnew
        ps = self.cur_ps
        for k in range(k0, k1):
            nc_.tensor.matmul(
                ps[:],
                lhsT=self.wsb[:, k, ts(m, 128)],
                rhs=self.rhs_tiles[k],
                start=(k == 0),
                stop=(k == self.ki - 1),
            )
        self.phase += 1
        if self.phase < self.split:
            return True
        self.phase = 0
        self.pos += 1
        # r,z chunks (m<8) are pre-scaled by WSCALE so the S-phase can
        # descale the whole PSUM (Whh fp8 part + injected gx) at once.
        # gbias for m<8 comes pre-scaled from the host.
        out = self.stg.tile([128, NCOL], BF16)
        sc = WSCALE if m < 2 * F else 1.0
        if m % 2 == 0:
            nc_.scalar.activation(out[:], ps[:], AF.Identity,
                                  bias=self.gb[:, m: m + 1], scale=sc)
        else:
            nc_.vector.tensor_scalar(out[:], ps[:], sc, self.gb[:, m: m + 1],
                                     ALU.mult, ALU.add)
        nc_.sync.dma_start(
            self.gx_r[:, ts(c, self.nub), m, :],
            out[:].rearrange("p (i c) -> p i c", c=TBLK * BQ),
        )
        return True

    def emit_chunks(self, n):
        for _ in range(n * M12 * self.split):
            self.emit_tile()

    def finish(self):
        while self.emit_tile():
            pass


def _s_phase(ctx, tc, nc, whhT_dram, nbias_dram, gx_dram, layer, y0own,
             y1T_dram, ident_dram, y0ex_chunks, ag_fn=None,
             interleave_fn=None):
    """512-step GRU scan.

    ag_fn(chunk) is called right after the last y0ex block of `chunk` is
    written, so the pairwise AllGather for that chunk overlaps the scan.
    interleave_fn(u) is called once per step to emit one P-phase tile into
    the PE queue (filling the scan's idle PE slots)."""
    nc_ = nc
    tag = f"s{layer}"
    wpool = ctx.enter_context(tc.tile_pool(name=f"whh_{tag}", bufs=1))
    cpool = ctx.enter_context(tc.tile_pool(name=f"c_{tag}", bufs=1))
    gxp = ctx.enter_context(tc.tile_pool(name=f"gx_{tag}", bufs=2))
    psum = ctx.enter_context(tc.tile_pool(name=f"ps_{tag}", bufs=1, space="PSUM"))
    gp = ctx.enter_context(tc.tile_pool(name=f"g_{tag}", bufs=2))
    yp = ctx.enter_context(tc.tile_pool(name=f"y_{tag}", bufs=2))

    whh = wpool.tile([128, F, G], FP8)
    nc_.sync.dma_start(whh[:], whhT_dram.ap().rearrange("(k p) g -> p k g", p=128))
    ident = cpool.tile([128, 128], FP8)
    nc_.sync.dma_start(ident[:], ident_dram.ap())
    # nbias comes pre-broadcast (and pre-scaled by WSCALE) from the host
    nbx = cpool.tile([128, F, BQ], BF16)
    nc_.sync.dma_start(nbx[:], nbias_dram.ap().rearrange("p (f b) -> p f b", b=BQ))
    zero_bf = cpool.tile([128, F, BQ], BF16)
    nc_.vector.memset(zero_bf[:], 0.0)

    gx_r = gx_dram.ap().rearrange("p (blk m c) -> p blk m c", m=M12, c=TBLK * BQ)
    y1_r = None
    if y1T_dram is not None:
        y1_r = y1T_dram.ap().rearrange("(f p) c -> p f c", p=128)

    def load_block(blk):
        t = gxp.tile([128, M12, TBLK * BQ], BF16)
        nc_.sync.dma_start(t[:], gx_r[:, blk, :, :])
        return t

    def write_block(wb, y1sb):
        if layer == 0:
            chunk = ((S - 1 - wb * TBLK) * BQ) // SBC
            y0e = y0ex_chunks[chunk].ap()
            lo = (S - 1 - wb * TBLK) * BQ - chunk * SBC
            for f in range(F):
                dst = bass.AP(
                    tensor=y0e.tensor,
                    offset=f * 128 * SBC + lo,
                    ap=[[SBC, 128], [-BQ, TBLK], [1, BQ]],
                )
                src = y0own[:, f, ts(wb, TBLK * BQ)].rearrange(
                    "p (t b) -> p t b", b=BQ)
                nc_.sync.dma_start(dst, src)
        else:
            nc_.sync.dma_start(y1_r[:, :, ts(wb, TBLK * BQ)], y1sb[:])

    # scan state
    hm1 = zero_bf[:]
    gxb_cur = load_block(0)
    gxb_next = None
    y1sb_cur = None

    # tile orders inside the PE burst: r,z consume h halves in order so the
    # next step can start as soon as the low half of h lands; the n group is
    # plain fold-major.
    zr_order = [(f, k) for f in range(F) for k in (0, 1)] + \
               [(f, k) for f in range(F) for k in (2, 3)]
    n_half = [[(f, k) for f in (0, 1) for k in range(F)],
              [(f - 2, k) for f in (2, 3) for k in range(F)]]

    for u in range(S):
        blk, j = divmod(u, TBLK)
        if j == 0:
            if u > 0:
                gxb_cur = gxb_next
            if blk + 1 < NB:
                gxb_next = load_block(blk + 1)
            if layer == 1:
                y1sb_cur = yp.tile([128, F, TBLK * BQ], BF16, tag="y1sb")

        # ---- PE burst, gate group order r, z, n0, n1.  The n group is in
        # two fold halves so the tanh chain starts while n1 matmuls run. ----
        def ps_tile(tg, nf):
            pst = psum.tile([128, nf, BQ], F32, tag=tg)
            return pst

        psr = ps_tile("r", F)
        psz = ps_tile("z", F)
        psn0 = ps_tile("n0", 2)
        psn1 = ps_tile("n1", 2)

        for gate, ps, order, inj, m0 in (
            ("r", psr, zr_order, gxb_cur[:, 0:F, ts(j, BQ)], 0),
            ("z", psz, zr_order, gxb_cur[:, F: 2 * F, ts(j, BQ)], F),
            ("n0", psn0, n_half[0], nbx[:, 0:2, :], 2 * F),
            ("n1", psn1, n_half[1], nbx[:, 2:4, :], 2 * F + 2),
        ):
            nc_.tensor.matmul(ps[:], lhsT=ident[:], rhs=inj,
                              start=True, stop=False, skip_group_check=True)
            last = order[-1]
            for (f, k) in order:
                nc_.tensor.matmul(ps[:, f, :],
                                  lhsT=whh[:, k, ts(m0 + f, 128)],
                                  rhs=hm1[:, k, :],
                                  start=False, stop=((f, k) == last),
                                  skip_group_check=True)

        # ---- gate math; critical chain per half: t1 -> t2 -> tanh -> m -> h.
        # ACT queue: sig_r, sig_z, omz, tanh0, tanh1.
        # DVE queue: t1h0, t2h0, t1h1, t2h1, p1, m0, h0, m1, h1. ----
        r = gp.tile([128, F, BQ], F32, tag="r")
        nc_.scalar.activation(r[:], psr[:], AF.Sigmoid, scale=SINV)
        z = gp.tile([128, F, BQ], F32, tag="z")
        nc_.scalar.activation(z[:], psz[:], AF.Sigmoid, scale=SINV)
        omz = gp.tile([128, F, BQ], F32, tag="omz")
        nc_.scalar.activation(omz[:], psz[:], AF.Sigmoid, scale=-SINV)

        t2 = gp.tile([128, F, BQ], F32, tag="t2")
        n = gp.tile([128, F, BQ], F32, tag="n")
        for hh, psn in ((0, psn0), (1, psn1)):
            sl = ts(hh, 2)
            t1 = gp.tile([128, 2, BQ], F32, tag=f"t1{hh}")
            nc_.vector.scalar_tensor_tensor(t1[:], psn[:], SINV, r[:, sl, :],
                                            ALU.mult, ALU.mult)
            nc_.vector.tensor_tensor(
                t2[:, sl, :], t1[:],
                gxb_cur[:, 2 * F + 2 * hh: 2 * F + 2 * hh + 2, ts(j, BQ)],
                ALU.add)
            nc_.scalar.activation(n[:, sl, :], t2[:, sl, :], AF.Tanh)

        if layer == 0:
            hslot = y0own[:, :, ts(u, BQ)]
        else:
            hslot = y1sb_cur[:, :, ts(j, BQ)]
        p1 = gp.tile([128, F, BQ], F32, tag="p1")
        nc_.vector.tensor_tensor(p1[:], z[:], hm1, ALU.mult)
        m = gp.tile([128, F, BQ], F32, tag="m")
        for hh in (0, 1):
            sl = ts(hh, 2)
            nc_.vector.tensor_tensor(m[:, sl, :], omz[:, sl, :], n[:, sl, :],
                                     ALU.mult)
            nc_.vector.tensor_tensor(hslot[:, sl, :], m[:, sl, :],
                                     p1[:, sl, :], ALU.add)
        hm1 = hslot

        if interleave_fn is not None:
            interleave_fn(u)
        if j == TBLK - 1:
            write_block(blk, y1sb_cur)
            if ag_fn is not None and (blk + 1) % NBC == 0:
                ag_fn(((S - 1 - blk * TBLK) * BQ) // SBC)


def build_program(debug=False):
    nc = bacc.Bacc("TRN2", target_bir_lowering=False, debug=debug,
                   num_devices=NCORE)

    def din(name, shape, dt):
        return nc.dram_tensor(name, list(shape), dt, kind="ExternalInput")

    xT = din("xT", (I, SB), BF16)
    wih0T = din("wih0T", (I, G), BF16)
    whh0T = din("whh0T", (H, G), FP8)
    wih1T = din("wih1T", (2 * H, G), BF16)
    whh1T = din("whh1T", (H, G), FP8)
    gbias0 = din("gbias0", (128, M12), F32)
    gbias1 = din("gbias1", (128, M12), F32)
    nbias0 = din("nbias0", (128, F * BQ), BF16)
    nbias1 = din("nbias1", (128, F * BQ), BF16)
    ident = din("ident", (128, 128), FP8)

    y1T = nc.dram_tensor("y1T", [H, SB], BF16, kind="ExternalOutput")

    gx0T = nc.dram_tensor("gx0T", [128, NB * M12 * TBLK * BQ], BF16)
    gx1T = nc.dram_tensor("gx1T", [128, NB * M12 * TBLK * BQ], BF16)
    y0ex_chunks = [nc.dram_tensor(f"y0ex{c}", [H, SBC], BF16)
                   for c in range(NCH)]
    y0g_chunks = [nc.dram_tensor(f"y0g{c}", [2, H, SBC], BF16)
                  for c in range(NCH)]
    y0loc_chunks = [nc.dram_tensor(f"y0loc{c}", [H, SBC], BF16)
                    for c in range(NCH)]

    groups = [[2 * q, 2 * q + 1] for q in range(4)]

    with tile.TileContext(nc) as tc:
        with ExitStack() as ctx:
            # ---- P0 stream: layer-0 input projection, head chunks up front,
            # the rest interleaved one tile per S0 step ----
            xpool = ctx.enter_context(tc.tile_pool(name="xsb", bufs=1))
            xsb = xpool.tile([128, KI0, SB], BF16)
            nc.sync.dma_start(xsb[:], xT.ap().rearrange("(k p) c -> p k c", p=128))
            y0pool = ctx.enter_context(tc.tile_pool(name="y0own", bufs=1))
            y0own = y0pool.tile([128, F, SB], BF16)
            with ExitStack() as p0ctx:
                p0 = PStream(p0ctx, tc, nc, wih0T, gbias0, gx0T, KI0,
                             lambda c: [xsb[:, k, ts(c, NCOL)] for k in range(KI0)],
                             "p0", psum_bufs=2)
                p0.emit_chunks(2)

                # ---- S0 scan (+ interleaved P0 tiles); y0own holds the h
                # sequence in SBUF.  Pairwise AllGathers fire per chunk. ----
                rank = nc.gpsimd.cc_rank(groups)

                def ag_fn(c):
                    nc.gpsimd.collective_compute(
                        "AllGather", ALU.bypass,
                        ins=[y0ex_chunks[c].ap()], outs=[y0g_chunks[c].ap()],
                        replica_groups=groups,
                    )
                    with tc.If(rank < 1) as cmp:
                        for rr in range(4):
                            nc.gpsimd.dma_start(
                                y0loc_chunks[c].ap()[ts(rr, 128), :],
                                y0g_chunks[c].ap()[1, ts(rr, 128), :])
                    with cmp.Else():
                        for rr in range(4):
                            nc.gpsimd.dma_start(
                                y0loc_chunks[c].ap()[ts(rr, 128), :],
                                y0g_chunks[c].ap()[0, ts(rr, 128), :])

                with ExitStack() as sctx:
                    _s_phase(sctx, tc, nc, whh0T, nbias0, gx0T, 0, y0own, None,
                             ident, y0ex_chunks, ag_fn=ag_fn,
                             interleave_fn=lambda u: p0.emit_tile())
                p0.finish()

            # ---- P1 stream: head chunks serial (waits on AG chunk 0), the
            # rest interleaved one tile per S1 step ----
            with ExitStack() as p1ctx:
                ppool = p1ctx.enter_context(tc.tile_pool(name="part", bufs=3))
                y0l_r = [t.ap().rearrange("(k p) c -> p k c", p=128)
                         for t in y0loc_chunks]
                cpc = SBC // NCOL  # NCOL chunks per AG chunk

                def rhs1(c):
                    part = ppool.tile([128, F, NCOL], BF16)
                    ch, off = divmod(c, cpc)
                    nc.sync.dma_start(part[:], y0l_r[ch][:, :, ts(off, NCOL)])
                    return [y0own[:, k, ts(c, NCOL)] for k in range(F)] + \
                           [part[:, k, :] for k in range(F)]

                # head=2 chunks: with split=2 the interleaved chunk c is
                # emitted by step 24(c-2)+24, ahead of the gx block prefetch
                # emitted at step 32c-16 (Tile deps follow tape order).
                p1 = PStream(p1ctx, tc, nc, wih1T, gbias1, gx1T, KI1, rhs1,
                             "p1", psum_bufs=2, split=2)
                p1.emit_chunks(2)

                # ---- S1: layer-1 scan (+ interleaved P1 tiles) -> y1T ----
                with ExitStack() as sctx:
                    _s_phase(sctx, tc, nc, whh1T, nbias1, gx1T, 1, None, y1T,
                             ident, None,
                             interleave_fn=lambda u: p1.emit_tile())
                p1.finish()

    nc.compile()
    return nc


_PROGRAM_CACHE = {}


def _get_program():
    if "nc" not in _PROGRAM_CACHE:
        _PROGRAM_CACHE["nc"] = build_program()
    return _PROGRAM_CACHE["nc"]


def _host_inputs(inputs):
    """Build the 8 per-core input maps from the full problem inputs."""
    bf = ml_dtypes.bfloat16
    f8 = ml_dtypes.float8_e4m3
    x = np.asarray(inputs["input"], np.float32)            # (S, B, I)
    in_maps = []
    for c in range(NCORE):
        fwd = c % 2 == 0
        q = c // 2
        d = "f" if fwd else "b"
        xq = x[:, q * BQ:(q + 1) * BQ, :]
        if not fwd:
            xq = xq[::-1]
        xTv = np.ascontiguousarray(xq.transpose(2, 0, 1).reshape(I, SB))

        def wT(wname):
            return np.ascontiguousarray(np.asarray(inputs[wname], np.float32).T)

        wih0 = wT(f"Wih_{d}0")        # (I, G)
        whh0 = wT(f"Whh_{d}0")        # (H, G)
        wih1_full = wT(f"Wih_{d}1")   # (2H, G); rows = y0 features [hf | hb]
        own_sl = slice(0, H) if fwd else slice(H, 2 * H)
        par_sl = slice(H, 2 * H) if fwd else slice(0, H)
        wih1 = np.concatenate([wih1_full[own_sl], wih1_full[par_sl]], axis=0)
        whh1 = wT(f"Whh_{d}1")

        def gbias(layer):
            bih = np.asarray(inputs[f"bih_{d}{layer}"], np.float32)
            bhh = np.asarray(inputs[f"bhh_{d}{layer}"], np.float32)
            gb = np.concatenate([bih[:2 * H] + bhh[:2 * H], bih[2 * H:]])
            gb = np.ascontiguousarray(gb.reshape(M12, 128).T)  # [128, M12]
            gb[:, : 2 * F] *= WSCALE   # r,z chunks pre-scaled (see _p_phase)
            return gb

        def nbias(layer):
            bhh = np.asarray(inputs[f"bhh_{d}{layer}"], np.float32)
            nb = (bhh[2 * H:] * WSCALE).reshape(F, 128).T  # [128, F], scaled
            return np.ascontiguousarray(
                np.broadcast_to(nb[:, :, None], (128, F, BQ)).reshape(
                    128, F * BQ)).astype(bf)

        in_maps.append({
            "xT": xTv.astype(bf),
            "wih0T": wih0.astype(bf),
            "whh0T": (whh0 * WSCALE).astype(f8),
            "wih1T": wih1.astype(bf),
            "whh1T": (whh1 * WSCALE).astype(f8),
            "gbias0": gbias(0), "gbias1": gbias(1),
            "nbias0": nbias(0), "nbias1": nbias(1),
            "ident": np.eye(128).astype(f8),
        })
    return in_maps


def kernel(**inputs) -> np.ndarray:
    nc = _get_program()
    in_maps = _host_inputs(inputs)
    trace = bool(int(os.environ.get("BIGRU_TRACE", "0")))
    kw = {}
    if trace and os.environ.get("BIGRU_TRACE_DIR"):
        kw["tmpdir"] = os.environ["BIGRU_TRACE_DIR"]
    res = run_bass_kernel_spmd(nc, in_maps, list(range(NCORE)), trace=trace, **kw)
    if trace and res.exec_time_ns is not None:
        print(f"HW exec time: {res.exec_time_ns} ns")
        _PROGRAM_CACHE["exec_time_ns"] = res.exec_time_ns
        _PROGRAM_CACHE["profile_json"] = res.profile_json

    out = np.empty((S, B, 2 * H), np.float32)
    for c in range(NCORE):
        fwd = c % 2 == 0
        q = c // 2
        y = np.asarray(res.results[c]["y1T"], dtype=np.float32)
        y = y.reshape(H, S, BQ).transpose(1, 2, 0)  # (S, BQ, H)
        if not fwd:
            y = y[::-1]
        out[:, q * BQ:(q + 1) * BQ, (0 if fwd else H):(H if fwd else 2 * H)] = y
    return out


# revision 45
# speedup vs baseline: 1.0991x; 1.0132x over previous
"""BiGRU (S=512, B=64, I=256, H=512, L=2) Trainium2 Bass kernel.

Strategy: 4-way batch split x 2-way direction split across 8 NeuronCores.
Cores 0-3 run the forward GRU chain (layers 0 and 1) for batch quarters
0-3; cores 4-7 run the backward chain (fed time-reversed input, so the
device program is identical on every core).  Per layer each core does:

  P-stream: gxT = Wih @ xT + bias (bf16 weights stationary, N=512 moving
           chunks), written to DRAM in a scan-blocked layout
           [128, NB, M12, TBLK*BQ].  Only a 2-chunk head runs up front;
           the remaining tiles are interleaved ONE PER SCAN STEP into the
           scan's idle PE slots, so the projection costs ~no wall time.
  S-phase: 512-step sequential GRU scan.  Whh is fp8-e4m3 (globally
           scaled; descale folded into activation `scale` operands), so
           the 48 LDWEIGHTS+MATMUL pairs per step run at ~20ns each.
           gx arrives via 16-step blocked prefetch (large DMA descriptors).
           Gate PSUM groups r, z, n0, n1 (n in fold-halves so the tanh
           chain starts while the n1 matmuls still run):
             r = sig(SINV*psr)  z = sig(SINV*psz)  omz = sig(-SINV*psz)
             per half: t = tanh((SINV*psn)*r + gxn);  m = omz*t
             h = m + z*h_prev   (low half first: next burst starts on it;
           the z/r matmul k-order consumes h low-half before high-half)

Between layers the forward/backward partners exchange their hidden-state
sequences with pairwise AllGathers split into 8 time-chunks, each fired
as soon as its chunk of y0ex is written during the scan (overlapping the
collective with the scan).  Final un-transpose / un-reverse of the
output happens on the host.
"""

import os
import sys
import numpy as np

for _p in ("/opt/trn_rl_repo", "/root/.axon_site/_ro/trn_rl_repo"):
    if os.path.isdir(_p) and _p not in sys.path:
        sys.path.insert(0, _p)

import ml_dtypes
from contextlib import ExitStack

import concourse.bass as bass
import concourse.tile as tile
from concourse import bacc, mybir
from concourse.bass import ts
from concourse.bass_utils import run_bass_kernel_spmd

BF16 = mybir.dt.bfloat16
FP8 = mybir.dt.float8e4
F32 = mybir.dt.float32
AF = mybir.ActivationFunctionType
ALU = mybir.AluOpType

# Whh is stored in fp8-e4m3 scaled so max|W| -> 240; the descale folds into
# the activation `scale` operand (gates) / one fused scalar_tensor_tensor (n).
WSCALE = float(240.0 * np.sqrt(512.0))
SINV = float(1.0 / WSCALE)

S, B, I, H, L = 512, 64, 256, 512, 2
G = 3 * H            # 1536 gate rows (r, z, n)
NCORE = 8
BQ = B // 4          # 16 batch per core
SB = S * BQ          # 8192 moving columns
F = H // 128         # 4 h-fold chunks
M12 = G // 128       # 12 gate chunks
KI0 = I // 128       # 2 contraction chunks, layer-0 input proj
KI1 = 2 * H // 128   # 8 contraction chunks, layer-1 input proj
NCOL = 512           # P-phase moving chunk width
TBLK = 16            # gx prefetch / y writeback block (steps)
NB = S // TBLK       # 32 blocks
NCH = 8              # AllGather chunks
NBC = NB // NCH      # blocks per chunk
SBC = SB // NCH      # columns per chunk
UB = NCOL // BQ      # steps covered by one P-phase column chunk (32)


class PStream:
    """Input-projection tile stream: gx = W @ xT + bias, bf16, written to
    gx_dram in blocked layout [128, NB, M12, TBLK*BQ].  Tiles can be emitted
    in bulk (emit_chunks/finish) or one at a time (emit_tile) so they
    interleave into the scan's idle PE slots."""

    def __init__(self, ctx, tc, nc, wT_dram, gbias_dram, gx_dram, ki, rhs_fn,
                 tag, psum_bufs=4, split=1):
        self.nc = nc
        self.ki = ki
        self.rhs_fn = rhs_fn
        self.split = split   # emit_tile() calls per tile (k-loop halves)
        self.phase = 0
        self.cur_ps = None
        self.wpool = ctx.enter_context(tc.tile_pool(name=f"w_{tag}", bufs=1))
        self.bpool = ctx.enter_context(tc.tile_pool(name=f"b_{tag}", bufs=1))
        self.psum = ctx.enter_context(
            tc.tile_pool(name=f"ps_{tag}", bufs=psum_bufs, space="PSUM"))
        self.stg = ctx.enter_context(tc.tile_pool(name=f"st_{tag}", bufs=4))

        self.wsb = self.wpool.tile([128, ki, G], BF16)
        nc.sync.dma_start(self.wsb[:],
                          wT_dram.ap().rearrange("(k p) g -> p k g", p=128))
        self.gb = self.bpool.tile([128, M12], F32)
        nc.sync.dma_start(self.gb[:], gbias_dram.ap())

        self.gx_r = gx_dram.ap().rearrange("p (blk m c) -> p blk m c",
                                           m=M12, c=TBLK * BQ)
        self.nub = NCOL // (TBLK * BQ)  # u-blocks per column chunk (2)
        self.tiles = [(c, m) for c in range(SB // NCOL) for m in range(M12)]
        self.pos = 0
        self.rhs_tiles = None

    def emit_tile(self):
        """Emit 1/split of one projection tile's matmuls (one call per scan
        step keeps the inserted PE work within the scan's idle slot)."""
        if self.pos >= len(self.tiles):
            return False
        nc_ = self.nc
        c, m = self.tiles[self.pos]
        k0 = self.phase * self.ki // self.split
        k1 = (self.phase + 1) * self.ki // self.split
        if k0 == 0:
            if m == 0:
                self.rhs_tiles = self.rhs_fn(c)  # ki [128, NCOL] bf16 APs
            pnew = self.psum.tile([128, NCOL], F32)
            self.cur_ps = pnew
        ps = self.cur_ps
        for k in range(k0, k1):
            nc_.tensor.matmul(
                ps[:],
                lhsT=self.wsb[:, k, ts(m, 128)],
                rhs=self.rhs_tiles[k],
                start=(k == 0),
                stop=(k == self.ki - 1),
            )
        self.phase += 1
        if self.phase < self.split:
            return True
        self.phase = 0
        self.pos += 1
        # r,z chunks (m<8) are pre-scaled by WSCALE so the S-phase can
        # descale the whole PSUM (Whh fp8 part + injected gx) at once.
        # gbias for m<8 comes pre-scaled from the host.
        out = self.stg.tile([128, NCOL], BF16)
        sc = WSCALE if m < 2 * F else 1.0
        if m % 2 == 0:
            nc_.scalar.activation(out[:], ps[:], AF.Identity,
                                  bias=self.gb[:, m: m + 1], scale=sc)
        else:
            nc_.vector.tensor_scalar(out[:], ps[:], sc, self.gb[:, m: m + 1],
                                     ALU.mult, ALU.add)
        nc_.sync.dma_start(
            self.gx_r[:, ts(c, self.nub), m, :],
            out[:].rearrange("p (i c) -> p i c", c=TBLK * BQ),
        )
        return True

    def emit_chunks(self, n):
        for _ in range(n * M12 * self.split):
            self.emit_tile()

    def finish(self):
        while self.emit_tile():
            pass


def _s_phase(ctx, tc, nc, whhT_dram, nbias_dram, gx_dram, layer, y0own,
             y1T_dram, ident_dram, y0ex_chunks, ag_fn=None,
             interleave_fn=None):
    """512-step GRU scan.

    ag_fn(chunk) is called right after the last y0ex block of `chunk` is
    written, so the pairwise AllGather for that chunk overlaps the scan.
    interleave_fn(u) is called once per step to emit one P-phase tile into
    the PE queue (filling the scan's idle PE slots)."""
    nc_ = nc
    tag = f"s{layer}"
    wpool = ctx.enter_context(tc.tile_pool(name=f"whh_{tag}", bufs=1))
    cpool = ctx.enter_context(tc.tile_pool(name=f"c_{tag}", bufs=1))
    gxp = ctx.enter_context(tc.tile_pool(name=f"gx_{tag}", bufs=2))
    psum = ctx.enter_context(tc.tile_pool(name=f"ps_{tag}", bufs=1, space="PSUM"))
    gp = ctx.enter_context(tc.tile_pool(name=f"g_{tag}", bufs=2))
    yp = ctx.enter_context(tc.tile_pool(name=f"y_{tag}", bufs=2))

    whh = wpool.tile([128, F, G], FP8)
    nc_.sync.dma_start(whh[:], whhT_dram.ap().rearrange("(k p) g -> p k g", p=128))
    ident = cpool.tile([128, 128], FP8)
    nc_.sync.dma_start(ident[:], ident_dram.ap())
    # nbias comes pre-broadcast (and pre-scaled by WSCALE) from the host
    nbx = cpool.tile([128, F, BQ], BF16)
    nc_.sync.dma_start(nbx[:], nbias_dram.ap().rearrange("p (f b) -> p f b", b=BQ))
    zero_bf = cpool.tile([128, F, BQ], BF16)
    nc_.vector.memset(zero_bf[:], 0.0)

    gx_r = gx_dram.ap().rearrange("p (blk m c) -> p blk m c", m=M12, c=TBLK * BQ)
    y1_r = None
    if y1T_dram is not None:
        y1_r = y1T_dram.ap().rearrange("(f p) c -> p f c", p=128)

    def load_block(blk):
        t = gxp.tile([128, M12, TBLK * BQ], BF16)
        nc_.sync.dma_start(t[:], gx_r[:, blk, :, :])
        return t

    def write_block(wb, y1sb):
        if layer == 0:
            chunk = ((S - 1 - wb * TBLK) * BQ) // SBC
            y0e = y0ex_chunks[chunk].ap()
            lo = (S - 1 - wb * TBLK) * BQ - chunk * SBC
            for f in range(F):
                dst = bass.AP(
                    tensor=y0e.tensor,
                    offset=f * 128 * SBC + lo,
                    ap=[[SBC, 128], [-BQ, TBLK], [1, BQ]],
                )
                src = y0own[:, f, ts(wb, TBLK * BQ)].rearrange(
                    "p (t b) -> p t b", b=BQ)
                nc_.sync.dma_start(dst, src)
        else:
            nc_.sync.dma_start(y1_r[:, :, ts(wb, TBLK * BQ)], y1sb[:])

    # scan state
    hm1 = zero_bf[:]
    gxb_cur = load_block(0)
    gxb_next = None
    y1sb_cur = None

    # tile orders inside the PE burst: r,z consume h halves in order so the
    # next step can start as soon as the low half of h lands; the n group is
    # plain fold-major.
    zr_order = [(f, k) for f in range(F) for k in (0, 1)] + \
               [(f, k) for f in range(F) for k in (2, 3)]
    n_half = [[(f, k) for f in (0, 1) for k in range(F)],
              [(f - 2, k) for f in (2, 3) for k in range(F)]]

    for u in range(S):
        blk, j = divmod(u, TBLK)
        if j == 0:
            if u > 0:
                gxb_cur = gxb_next
            if blk + 1 < NB:
                gxb_next = load_block(blk + 1)
            if layer == 1:
                y1sb_cur = yp.tile([128, F, TBLK * BQ], BF16, tag="y1sb")

        # ---- PE burst, gate group order r, z, n0, n1.  The n group is in
        # two fold halves so the tanh chain starts while n1 matmuls run. ----
        def ps_tile(tg, nf):
            pst = psum.tile([128, nf, BQ], F32, tag=tg)
            return pst

        psr = ps_tile("r", F)
        psz = ps_tile("z", F)
        psn0 = ps_tile("n0", 2)
        psn1 = ps_tile("n1", 2)

        for gate, ps, order, inj, m0 in (
            ("r", psr, zr_order, gxb_cur[:, 0:F, ts(j, BQ)], 0),
            ("z", psz, zr_order, gxb_cur[:, F: 2 * F, ts(j, BQ)], F),
            ("n0", psn0, n_half[0], nbx[:, 0:2, :], 2 * F),
            ("n1", psn1, n_half[1], nbx[:, 2:4, :], 2 * F + 2),
        ):
            nc_.tensor.matmul(ps[:], lhsT=ident[:], rhs=inj,
                              start=True, stop=False, skip_group_check=True)
            last = order[-1]
            for (f, k) in order:
                nc_.tensor.matmul(ps[:, f, :],
                                  lhsT=whh[:, k, ts(m0 + f, 128)],
                                  rhs=hm1[:, k, :],
                                  start=False, stop=((f, k) == last),
                                  skip_group_check=True)

        # ---- gate math; critical chain per half: t1 -> t2 -> tanh -> m -> h.
        # ACT queue: sig_r, sig_z, omz, tanh0, tanh1.
        # DVE queue: t1h0, t2h0, t1h1, t2h1, p1, m0, h0, m1, h1. ----
        # bf16 gate tensors: 2-byte packed operands enable the DVE 2x mode
        # on the t2/p1/m/h ops (r stays f32; t1 reads f32 PSUM anyway).
        r = gp.tile([128, F, BQ], F32, tag="r")
        nc_.scalar.activation(r[:], psr[:], AF.Sigmoid, scale=SINV)
        z = gp.tile([128, F, BQ], BF16, tag="z")
        nc_.scalar.activation(z[:], psz[:], AF.Sigmoid, scale=SINV)
        omz = gp.tile([128, F, BQ], BF16, tag="omz")
        nc_.scalar.activation(omz[:], psz[:], AF.Sigmoid, scale=-SINV)

        t2 = gp.tile([128, F, BQ], BF16, tag="t2")
        n = gp.tile([128, F, BQ], BF16, tag="n")
        for hh, psn in ((0, psn0), (1, psn1)):
            sl = ts(hh, 2)
            t1 = gp.tile([128, 2, BQ], BF16, tag=f"t1{hh}")
            nc_.vector.scalar_tensor_tensor(t1[:], psn[:], SINV, r[:, sl, :],
                                            ALU.mult, ALU.mult)
            nc_.vector.tensor_tensor(
                t2[:, sl, :], t1[:],
                gxb_cur[:, 2 * F + 2 * hh: 2 * F + 2 * hh + 2, ts(j, BQ)],
                ALU.add)
            nc_.scalar.activation(n[:, sl, :], t2[:, sl, :], AF.Tanh)

        if layer == 0:
            hslot = y0own[:, :, ts(u, BQ)]
        else:
            hslot = y1sb_cur[:, :, ts(j, BQ)]
        p1 = gp.tile([128, F, BQ], BF16, tag="p1")
        nc_.vector.tensor_tensor(p1[:], z[:], hm1, ALU.mult)
        m = gp.tile([128, F, BQ], BF16, tag="m")
        for hh in (0, 1):
            sl = ts(hh, 2)
            nc_.vector.tensor_tensor(m[:, sl, :], omz[:, sl, :], n[:, sl, :],
                                     ALU.mult)
            nc_.vector.tensor_tensor(hslot[:, sl, :], m[:, sl, :],
                                     p1[:, sl, :], ALU.add)
        hm1 = hslot

        if interleave_fn is not None:
            interleave_fn(u)
        if j == TBLK - 1:
            write_block(blk, y1sb_cur)
            if ag_fn is not None and (blk + 1) % NBC == 0:
                ag_fn(((S - 1 - blk * TBLK) * BQ) // SBC)


def build_program(debug=False):
    nc = bacc.Bacc("TRN2", target_bir_lowering=False, debug=debug,
                   num_devices=NCORE)

    def din(name, shape, dt):
        return nc.dram_tensor(name, list(shape), dt, kind="ExternalInput")

    xT = din("xT", (I, SB), BF16)
    wih0T = din("wih0T", (I, G), BF16)
    whh0T = din("whh0T", (H, G), FP8)
    wih1T = din("wih1T", (2 * H, G), BF16)
    whh1T = din("whh1T", (H, G), FP8)
    gbias0 = din("gbias0", (128, M12), F32)
    gbias1 = din("gbias1", (128, M12), F32)
    nbias0 = din("nbias0", (128, F * BQ), BF16)
    nbias1 = din("nbias1", (128, F * BQ), BF16)
    ident = din("ident", (128, 128), FP8)

    y1T = nc.dram_tensor("y1T", [H, SB], BF16, kind="ExternalOutput")

    gx0T = nc.dram_tensor("gx0T", [128, NB * M12 * TBLK * BQ], BF16)
    gx1T = nc.dram_tensor("gx1T", [128, NB * M12 * TBLK * BQ], BF16)
    y0ex_chunks = [nc.dram_tensor(f"y0ex{c}", [H, SBC], BF16)
                   for c in range(NCH)]
    y0g_chunks = [nc.dram_tensor(f"y0g{c}", [2, H, SBC], BF16)
                  for c in range(NCH)]
    y0loc_chunks = [nc.dram_tensor(f"y0loc{c}", [H, SBC], BF16)
                    for c in range(NCH)]

    groups = [[2 * q, 2 * q + 1] for q in range(4)]

    with tile.TileContext(nc) as tc:
        with ExitStack() as ctx:
            # ---- P0 stream: layer-0 input projection, head chunks up front,
            # the rest interleaved one tile per S0 step ----
            xpool = ctx.enter_context(tc.tile_pool(name="xsb", bufs=1))
            xsb = xpool.tile([128, KI0, SB], BF16)
            nc.sync.dma_start(xsb[:], xT.ap().rearrange("(k p) c -> p k c", p=128))
            y0pool = ctx.enter_context(tc.tile_pool(name="y0own", bufs=1))
            y0own = y0pool.tile([128, F, SB], BF16)
            with ExitStack() as p0ctx:
                p0 = PStream(p0ctx, tc, nc, wih0T, gbias0, gx0T, KI0,
                             lambda c: [xsb[:, k, ts(c, NCOL)] for k in range(KI0)],
                             "p0", psum_bufs=2)
                p0.emit_chunks(2)

                # ---- S0 scan (+ interleaved P0 tiles); y0own holds the h
                # sequence in SBUF.  Pairwise AllGathers fire per chunk. ----
                rank = nc.gpsimd.cc_rank(groups)

                def ag_fn(c):
                    nc.gpsimd.collective_compute(
                        "AllGather", ALU.bypass,
                        ins=[y0ex_chunks[c].ap()], outs=[y0g_chunks[c].ap()],
                        replica_groups=groups,
                    )
                    with tc.If(rank < 1) as cmp:
                        for rr in range(4):
                            nc.gpsimd.dma_start(
                                y0loc_chunks[c].ap()[ts(rr, 128), :],
                                y0g_chunks[c].ap()[1, ts(rr, 128), :])
                    with cmp.Else():
                        for rr in range(4):
                            nc.gpsimd.dma_start(
                                y0loc_chunks[c].ap()[ts(rr, 128), :],
                                y0g_chunks[c].ap()[0, ts(rr, 128), :])

                with ExitStack() as sctx:
                    _s_phase(sctx, tc, nc, whh0T, nbias0, gx0T, 0, y0own, None,
                             ident, y0ex_chunks, ag_fn=ag_fn,
                             interleave_fn=lambda u: p0.emit_tile())
                p0.finish()

            # ---- P1 stream: head chunks serial (waits on AG chunk 0), the
            # rest interleaved one tile per S1 step ----
            with ExitStack() as p1ctx:
                ppool = p1ctx.enter_context(tc.tile_pool(name="part", bufs=3))
                y0l_r = [t.ap().rearrange("(k p) c -> p k c", p=128)
                         for t in y0loc_chunks]
                cpc = SBC // NCOL  # NCOL chunks per AG chunk

                def rhs1(c):
                    part = ppool.tile([128, F, NCOL], BF16)
                    ch, off = divmod(c, cpc)
                    nc.sync.dma_start(part[:], y0l_r[ch][:, :, ts(off, NCOL)])
                    return [y0own[:, k, ts(c, NCOL)] for k in range(F)] + \
                           [part[:, k, :] for k in range(F)]

                # head=2 chunks: with split=2 the interleaved chunk c is
                # emitted by step 24(c-2)+24, ahead of the gx block prefetch
                # emitted at step 32c-16 (Tile deps follow tape order).
                p1 = PStream(p1ctx, tc, nc, wih1T, gbias1, gx1T, KI1, rhs1,
                             "p1", psum_bufs=2, split=2)
                p1.emit_chunks(2)

                # ---- S1: layer-1 scan (+ interleaved P1 tiles) -> y1T ----
                with ExitStack() as sctx:
                    _s_phase(sctx, tc, nc, whh1T, nbias1, gx1T, 1, None, y1T,
                             ident, None,
                             interleave_fn=lambda u: p1.emit_tile())
                p1.finish()

    nc.compile()
    return nc


_PROGRAM_CACHE = {}


def _get_program():
    if "nc" not in _PROGRAM_CACHE:
        _PROGRAM_CACHE["nc"] = build_program()
    return _PROGRAM_CACHE["nc"]


def _host_inputs(inputs):
    """Build the 8 per-core input maps from the full problem inputs."""
    bf = ml_dtypes.bfloat16
    f8 = ml_dtypes.float8_e4m3
    x = np.asarray(inputs["input"], np.float32)            # (S, B, I)
    in_maps = []
    for c in range(NCORE):
        fwd = c % 2 == 0
        q = c // 2
        d = "f" if fwd else "b"
        xq = x[:, q * BQ:(q + 1) * BQ, :]
        if not fwd:
            xq = xq[::-1]
        xTv = np.ascontiguousarray(xq.transpose(2, 0, 1).reshape(I, SB))

        def wT(wname):
            return np.ascontiguousarray(np.asarray(inputs[wname], np.float32).T)

        wih0 = wT(f"Wih_{d}0")        # (I, G)
        whh0 = wT(f"Whh_{d}0")        # (H, G)
        wih1_full = wT(f"Wih_{d}1")   # (2H, G); rows = y0 features [hf | hb]
        own_sl = slice(0, H) if fwd else slice(H, 2 * H)
        par_sl = slice(H, 2 * H) if fwd else slice(0, H)
        wih1 = np.concatenate([wih1_full[own_sl], wih1_full[par_sl]], axis=0)
        whh1 = wT(f"Whh_{d}1")

        def gbias(layer):
            bih = np.asarray(inputs[f"bih_{d}{layer}"], np.float32)
            bhh = np.asarray(inputs[f"bhh_{d}{layer}"], np.float32)
            gb = np.concatenate([bih[:2 * H] + bhh[:2 * H], bih[2 * H:]])
            gb = np.ascontiguousarray(gb.reshape(M12, 128).T)  # [128, M12]
            gb[:, : 2 * F] *= WSCALE   # r,z chunks pre-scaled (see _p_phase)
            return gb

        def nbias(layer):
            bhh = np.asarray(inputs[f"bhh_{d}{layer}"], np.float32)
            nb = (bhh[2 * H:] * WSCALE).reshape(F, 128).T  # [128, F], scaled
            return np.ascontiguousarray(
                np.broadcast_to(nb[:, :, None], (128, F, BQ)).reshape(
                    128, F * BQ)).astype(bf)

        in_maps.append({
            "xT": xTv.astype(bf),
            "wih0T": wih0.astype(bf),
            "whh0T": (whh0 * WSCALE).astype(f8),
            "wih1T": wih1.astype(bf),
            "whh1T": (whh1 * WSCALE).astype(f8),
            "gbias0": gbias(0), "gbias1": gbias(1),
            "nbias0": nbias(0), "nbias1": nbias(1),
            "ident": np.eye(128).astype(f8),
        })
    return in_maps


def kernel(**inputs) -> np.ndarray:
    nc = _get_program()
    in_maps = _host_inputs(inputs)
    trace = bool(int(os.environ.get("BIGRU_TRACE", "0")))
    kw = {}
    if trace and os.environ.get("BIGRU_TRACE_DIR"):
        kw["tmpdir"] = os.environ["BIGRU_TRACE_DIR"]
    res = run_bass_kernel_spmd(nc, in_maps, list(range(NCORE)), trace=trace, **kw)
    if trace and res.exec_time_ns is not None:
        print(f"HW exec time: {res.exec_time_ns} ns")
        _PROGRAM_CACHE["exec_time_ns"] = res.exec_time_ns
        _PROGRAM_CACHE["profile_json"] = res.profile_json

    out = np.empty((S, B, 2 * H), np.float32)
    for c in range(NCORE):
        fwd = c % 2 == 0
        q = c // 2
        y = np.asarray(res.results[c]["y1T"], dtype=np.float32)
        y = y.reshape(H, S, BQ).transpose(1, 2, 0)  # (S, BQ, H)
        if not fwd:
            y = y[::-1]
        out[:, q * BQ:(q + 1) * BQ, (0 if fwd else H):(H if fwd else 2 * H)] = y
    return out
